# revision 1
# baseline (speedup 1.0000x reference)
"""Trainium2 Bass kernel for nn_GAT_47906065220065.

SSGConv (K=1, alpha=0.5) -> GATv2(12 heads, 12 dim) -> GATv2(1 head, 64 dim)
over a fixed random graph: N=100000 nodes, E=1000000 edges (+ self loops).

Distribution: nodes are relabeled by a degree-balanced permutation, then
destination nodes are sharded contiguously across the 8 cores (12800 per
core).  Edges live with their destination core, grouped into 128-dst
blocks with near-equal edge counts.  Per edge-tile (128 edges):

  - source rows are fetched with indirect DMA (INDIRECT1D, 128 rows/inst)
  - a one-hot selection matrix S[e,d] = (dst_local(e)==d)*coeff is built
    with one tensor_scalar op against a constant iota tile
  - destination-side features are broadcast per edge with a PE matmul
    (lhsT = S^T), the gathered source rows are added with an
    identity-matmul, attention logits/softmax numerators are computed on
    DVE/ACT, and the per-block segment sums (numerator + denominator)
    accumulate in PSUM via a matmul with lhsT = S.

Softmax uses a constant shift (exp(l - 4)) instead of the per-segment max;
logits for this data are in [-5.3, 4.1] so this matches the reference to
float32 accuracy.  leaky_relu(z, 0.2) is composed as 0.6*z + 0.4*|z| with
the 0.6 folded into the right/W_r weights and a 0.6*identity matmul.

Layer outputs are stored feature-major (x^T) per core and AllGathered so
every core can run the next layer's dense projections; the gather tables
(xl = x @ W_l, node-major) are materialized per core.
"""

import os
import sys
import types

sys.path.insert(0, '/opt/trn_rl_repo')

import numpy as np

import bass_rust
import concourse.bacc as bacc
import concourse.bass as bass
import concourse.mybir as mybir
import concourse.tile as tile
from concourse.bass_utils import run_bass_kernel_spmd

# ---------------------------------------------------------------- sizes
N = 100000
NPAD = 102400
NCORES = 8
PERCORE = NPAD // NCORES          # 12800
NBLK = PERCORE // 128             # 100
NBINS = NPAD // 128               # 800
D_IN = 64
F1 = 144                          # heads1*dim1
H1, C1 = 12, 12
F2 = 64                           # heads2*dim2
SHIFT = 4.0                       # constant softmax shift (logits < 4.1)
PHASES = int(os.environ.get("KERNEL_PHASES", "5"))
NEG = 0.2
FP = mybir.dt.float32
I32 = mybir.dt.int32

AF = mybir.ActivationFunctionType
ALU = mybir.AluOpType


# ------------------------------------------------- walrus compatibility
def _drain_and_barrier_split(self, tick_clock, wait_clock):
    """End-of-TileContext drain for a walrus build that accepts at most
    one sync wait per instruction: spread the global-clock waits over
    NoOps instead of piling them on the final drain."""
    from concourse.vector_clock import ScopedClock

    carrier = self.nc.sync.nop(nofuse=True)
    wait_clock.add_sem_waits(
        carrier.ins, ScopedClock({None: tick_clock.global_clock})
    )
    si0 = carrier.ins.sync_info
    waits = list(si0.on_wait or []) if si0 is not None else []
    if len(waits) > 1:
        carrier.ins.sync_info = bass_rust.SyncInfo(
            on_wait=waits[:1], on_update=list(si0.on_update or [])
        )
        for w in waits[1:]:
            extra = self.nc.sync.nop(nofuse=True)
            extra.ins.sync_info = bass_rust.SyncInfo(on_wait=[w], on_update=[])
    self.nc.sync.drain()

    self.nc.all_engine_barrier()
    assert self.sems is not None
    popped = self.nc._tile_sem_poison_stack.pop()
    assert popped is self._sem_poison
    self.nc.clear_and_free_semaphores(list(self.sems.allocated().values()))
    self.nc.all_engine_barrier()


tile.TileContext._drain_and_barrier = _drain_and_barrier_split

_WSPLIT_N = [0]


def _split_sync_waits(nc):
    """Move extra sync waits (this walrus allows 1/instruction) onto NoOps
    inserted before the over-subscribed instruction on the same engine."""
    def make_nop(engine, wait):
        _WSPLIT_N[0] += 1
        return mybir.InstNoOp(
            name=f"WSPLIT-{_WSPLIT_N[0]}", opcode="NoOp", engine=engine,
            debug=None, ins=[], outs=[], descendants=None,
            sync_info=bass_rust.SyncInfo(on_wait=[wait], on_update=[]),
            bass_sim_breakpoint=False, bass_priority=0,
            bass_wait_until_ts=None, bass_scheduled_tick=None,
            bass_scheduled_proc=None, bass_scheduled_scope=None,
            bass_addl_debug=None, text_hint=None, bass_nofuse=True,
        )

    for f in nc.m.functions:
        for bb in f.blocks:
            if not any(
                inst.sync_info and inst.sync_info.on_wait
                and len(inst.sync_info.on_wait) > 1
                for inst in bb.instructions
            ):
                continue
            new_insts = []
            for inst in bb.instructions:
                si = inst.sync_info
                waits = list(si.on_wait) if si and si.on_wait else []
                if len(waits) > 1:
                    for w in waits[:-1]:
                        new_insts.append(make_nop(inst.engine, w))
                    inst.sync_info = bass_rust.SyncInfo(
                        on_wait=[waits[-1]], on_update=list(si.on_update or [])
                    )
                new_insts.append(inst)
            bb.instructions = new_insts


# ------------------------------------------------------------ host prep
def _host_prep(features, edge_index, params):
    x = np.ascontiguousarray(np.asarray(features), dtype=np.float32)
    ei = np.asarray(edge_index)
    src = ei[0].astype(np.int64)
    dst = ei[1].astype(np.int64)
    E = src.shape[0]

    s = np.concatenate([src, np.arange(N, dtype=np.int64)])
    d = np.concatenate([dst, np.arange(N, dtype=np.int64)])
    deg = np.bincount(d, minlength=N).astype(np.float32)
    dinv = 1.0 / np.sqrt(deg)
    norm = (dinv[s] * dinv[d]).astype(np.float32)

    # degree-balanced node relabeling: heaviest nodes round-robin over bins
    order = np.argsort(-deg, kind='stable')
    order_full = np.concatenate([order, np.arange(N, NPAD, dtype=np.int64)])
    newid = np.empty(NPAD, dtype=np.int64)
    ranks = np.arange(NPAD, dtype=np.int64)
    newid[order_full] = (ranks % NBINS) * 128 + ranks // NBINS

    xp = np.zeros((NPAD, D_IN), dtype=np.float32)
    xp[newid[:N]] = x

    sp = newid[s]
    dp = newid[d]
    blk = dp >> 7                  # global 128-dst block id (0..NBINS)
    dloc = (dp & 127).astype(np.float32)

    # group edges by block
    eorder = np.argsort(blk, kind='stable')
    sp, dloc_s, norm_s, blk_s = sp[eorder], dloc[eorder], norm[eorder], blk[eorder]
    counts = np.bincount(blk_s, minlength=NBINS)
    tiles_per_blk = (counts + 127) // 128
    T = int(tiles_per_blk.max())

    # padded per-block edge arrays [NBINS, T*128]
    cap = T * 128
    srcs_p = np.zeros((NBINS, cap), dtype=np.int32)
    dstf_p = np.zeros((NBINS, cap), dtype=np.float32)
    cnorm_p = np.zeros((NBINS, cap), dtype=np.float32)
    cval_p = np.zeros((NBINS, cap), dtype=np.float32)
    ofs = np.concatenate([[0], np.cumsum(counts)])
    within = np.arange(len(sp)) - ofs[blk_s]
    srcs_p[blk_s, within] = sp.astype(np.int32)
    dstf_p[blk_s, within] = dloc_s
    cnorm_p[blk_s, within] = norm_s
    cval_p[blk_s, within] = 1.0

    # stream layout [128, NBLK*T] per core: col b*T+t, row p -> edge (b, t*128+p)
    def streams(arr):
        a = arr.reshape(NCORES, NBLK, T, 128)
        return [np.ascontiguousarray(a[c].transpose(2, 0, 1).reshape(128, NBLK * T))
                for c in range(NCORES)]

    srcs_c = streams(srcs_p)
    dstf_c = streams(dstf_p)
    cnorm_c = streams(cnorm_p)
    cval_c = streams(cval_p)

    g = lambda k: np.ascontiguousarray(np.asarray(params[k]), dtype=np.float32)
    W_ssg, b_ssg = g('W_ssg'), g('b_ssg')
    W1l, b1l, W1r, b1r = g('W1l'), g('b1l'), g('W1r'), g('b1r')
    att1, bias1 = g('att1'), g('bias1')
    W2l, b2l, W2r, b2r = g('W2l'), g('b2l'), g('W2r'), g('b2r')
    att2, bias2 = g('att2'), g('bias2')

    col = lambda v: np.ascontiguousarray(v.reshape(-1, 1), dtype=np.float32)
    consts = dict(
        iota=np.tile(np.arange(128, dtype=np.float32), (128, 1)),
        ident=np.eye(128, dtype=np.float32),
        ident06=(0.6 * np.eye(128, dtype=np.float32)),
        wssg=0.5 * W_ssg,
        bssg=col(b_ssg),
        w1la=W1l[:, :128], w1lb=W1l[:, 128:],
        b1la=col(b1l[:128]), b1lb=col(b1l[128:]),
        w1ra=0.6 * W1r[:, :128], w1rb=0.6 * W1r[:, 128:],
        b1ra=col(0.6 * b1r[:128]), b1rb=col(0.6 * b1r[128:]),
        att1r=np.tile(att1.reshape(1, F1), (128, 1)),
        bias1a=col(bias1[:128]), bias1b=col(bias1[128:]),
        w2lt=W2l[:128, :], w2lb=W2l[128:, :], b2l=col(b2l),
        w2rt=0.6 * W2r[:128, :], w2rb=0.6 * W2r[128:, :], b2r=col(0.6 * b2r),
        att2r=np.tile(att2.reshape(1, F2), (128, 1)),
        nshift=np.full((128, 1), -SHIFT, dtype=np.float32),
        bias2r=np.tile(bias2.reshape(1, F2), (128, 1)),
    )
    consts = {k: np.ascontiguousarray(v, dtype=np.float32) for k, v in consts.items()}

    in_maps = []
    for c in range(NCORES):
        m = dict(consts)
        m['xg'] = xp
        m['xown'] = np.ascontiguousarray(xp[c * PERCORE:(c + 1) * PERCORE])
        m['srcs'] = srcs_c[c]
        m['dstf'] = dstf_c[c]
        m['cnorm'] = cnorm_c[c]
        m['cval'] = cval_c[c]
        in_maps.append(m)
    return in_maps, newid, T


# --------------------------------------------------------- kernel build
def _build(T):
    nc = bacc.Bacc()
    NT = NBLK * T

    xg = nc.declare_dram_parameter("xg", [NPAD, D_IN], FP, isOutput=False)
    xown = nc.declare_dram_parameter("xown", [PERCORE, D_IN], FP, isOutput=False)
    srcs = nc.declare_dram_parameter("srcs", [128, NT], I32, isOutput=False)
    dstf = nc.declare_dram_parameter("dstf", [128, NT], FP, isOutput=False)
    cnorm = nc.declare_dram_parameter("cnorm", [128, NT], FP, isOutput=False)
    cval = nc.declare_dram_parameter("cval", [128, NT], FP, isOutput=False)

    cshape = dict(
        iota=[128, 128], ident=[128, 128], ident06=[128, 128],
        wssg=[64, 64], bssg=[64, 1],
        w1la=[64, 128], w1lb=[64, 16], b1la=[128, 1], b1lb=[16, 1],
        w1ra=[64, 128], w1rb=[64, 16], b1ra=[128, 1], b1rb=[16, 1],
        att1r=[128, F1], bias1a=[128, 1], bias1b=[16, 1],
        w2lt=[128, 64], w2lb=[16, 64], b2l=[64, 1],
        w2rt=[128, 64], w2rb=[16, 64], b2r=[64, 1],
        att2r=[128, F2], bias2r=[128, F2], nshift=[128, 1],
    )
    cparams = {k: nc.declare_dram_parameter(k, v, FP, isOutput=False)
               for k, v in cshape.items()}

    out = nc.declare_dram_parameter("out", [PERCORE, F2], FP, isOutput=True)

    x1T_loc = nc.dram_tensor("x1T_loc", [D_IN, PERCORE], FP)
    x1T_all = nc.dram_tensor("x1T_all", [NCORES * D_IN, PERCORE], FP,
                             addr_space="Shared")
    y1T_loc = nc.dram_tensor("y1T_loc", [F1, PERCORE], FP)
    y1T_all = nc.dram_tensor("y1T_all", [NCORES * F1, PERCORE], FP,
                             addr_space="Shared")
    xl1 = nc.dram_tensor("xl1", [NPAD, F1], FP)
    xl2 = nc.dram_tensor("xl2", [NPAD, F2], FP)

    ds = bass.ds
    rg = [list(range(NCORES))]

    with tile.TileContext(nc) as tc:
        cpool = tc.alloc_tile_pool(name="consts", bufs=1)
        ct = {}
        for k, shp in cshape.items():
            ct[k] = cpool.tile(shp, FP, tag=f"c_{k}", name=f"c_{k}")
            nc.sync.dma_start(out=ct[k][:], in_=cparams[k][:])

        # ---------------- phase 1: SSG conv -> x1T_loc ----------------
        if True:
         with (tc.tile_pool(name="p1s", bufs=3) as pool,
              tc.tile_pool(name="p1a", bufs=2, space="PSUM") as ppa,
              tc.tile_pool(name="p1t", bufs=2, space="PSUM") as ppt):
            with tc.For_i(0, NBLK, 1) as b:
                stg_s = pool.tile([128, T], I32, tag="stg_s")
                stg_d = pool.tile([128, T], FP, tag="stg_d")
                stg_c = pool.tile([128, T], FP, tag="stg_c")
                nc.sync.dma_start(out=stg_s[:], in_=srcs[:, ds(b * T, T)])
                nc.sync.dma_start(out=stg_d[:], in_=dstf[:, ds(b * T, T)])
                nc.sync.dma_start(out=stg_c[:], in_=cnorm[:, ds(b * T, T)])
                agg = ppa.tile([128, D_IN], FP, tag="agg")
                for t in range(T):
                    gx = pool.tile([128, D_IN], FP, tag="gx")
                    nc.gpsimd.indirect_dma_start(
                        out=gx[:], out_offset=None, in_=xg[:],
                        in_offset=bass.IndirectOffsetOnAxis(
                            ap=stg_s[:, t:t + 1], axis=0))
                    S = pool.tile([128, 128], FP, tag="S")
                    nc.vector.tensor_scalar(
                        S[:], ct['iota'][:], stg_d[:, t:t + 1],
                        stg_c[:, t:t + 1], op0=ALU.is_equal, op1=ALU.mult)
                    nc.tensor.matmul(agg[:], lhsT=S[:], rhs=gx[:],
                                     start=(t == 0), stop=(t == T - 1))
                xo = pool.tile([128, D_IN], FP, tag="xo")
                nc.sync.dma_start(out=xo[:], in_=xown[ds(b * 128, 128), :])
                hs = pool.tile([128, D_IN], FP, tag="hs")
                nc.vector.tensor_tensor(out=hs[:], in0=xo[:], in1=agg[:],
                                        op=ALU.add)
                hT = ppt.tile([64, 128], FP, tag="p1t")
                nc.tensor.transpose(out=hT[:], in_=hs[:], identity=ct['ident'][:])
                hTs = pool.tile([64, 128], FP, tag="hTs")
                nc.scalar.activation(hTs[:], hT[:], AF.Copy)
                x1p = ppt.tile([64, 128], FP, tag="p1t")
                nc.tensor.matmul(x1p[:], lhsT=ct['wssg'][:], rhs=hTs[:],
                                 start=True, stop=True)
                x1s = pool.tile([64, 128], FP, tag="x1s")
                nc.scalar.activation(x1s[:], x1p[:], AF.Identity,
                                     bias=ct['bssg'][:, :1])
                nc.sync.dma_start(out=x1T_loc[:, ds(b * 128, 128)], in_=x1s[:])

        if PHASES >= 2:
         nc.gpsimd.collective_compute(
            "AllGather", ALU.bypass, replica_groups=rg,
            ins=[x1T_loc[:]], outs=[x1T_all[:]])

        # ---------------- phase 2: xl1 = x1 @ W1l + b1l --------------
        CH = PERCORE // 512  # 25
        if PHASES >= 2:
         with (tc.tile_pool(name="p2s", bufs=3) as pool,
              tc.tile_pool(name="p2p", bufs=2, space="PSUM") as pp):
            with tc.For_i(0, CH, 1) as ci:
                for r in range(NCORES):
                    rhs = pool.tile([64, 512], FP, tag="rhs")
                    nc.sync.dma_start(
                        out=rhs[:],
                        in_=x1T_all[r * 64:(r + 1) * 64, ds(ci * 512, 512)])
                    psA = pp.tile([128, 512], FP, tag="psA")
                    nc.tensor.matmul(psA[:], lhsT=ct['w1la'][:], rhs=rhs[:],
                                     start=True, stop=True)
                    psB = pp.tile([16, 512], FP, tag="psB")
                    nc.tensor.matmul(psB[:], lhsT=ct['w1lb'][:], rhs=rhs[:],
                                     start=True, stop=True)
                    sA = pool.tile([128, 512], FP, tag="sA")
                    nc.scalar.activation(sA[:], psA[:], AF.Identity,
                                         bias=ct['b1la'][:, :1])
                    sB = pool.tile([16, 512], FP, tag="sB")
                    nc.scalar.activation(sB[:], psB[:], AF.Identity,
                                         bias=ct['b1lb'][:, :1])
                    for gi in range(4):
                        tpA = pp.tile([128, 128], FP, tag="tp2")
                        nc.tensor.transpose(out=tpA[:],
                                            in_=sA[:, gi * 128:(gi + 1) * 128],
                                            identity=ct['ident'][:])
                        tpB = pp.tile([128, 16], FP, tag="tp2")
                        nc.tensor.transpose(out=tpB[:],
                                            in_=sB[:, gi * 128:(gi + 1) * 128],
                                            identity=ct['ident'][:16, :16])
                        onm = pool.tile([128, F1], FP, tag="onm")
                        nc.scalar.activation(onm[:, :128], tpA[:], AF.Copy)
                        nc.scalar.activation(onm[:, 128:F1], tpB[:], AF.Copy)
                        nc.sync.dma_start(
                            out=xl1[ds(r * PERCORE + ci * 512 + gi * 128, 128), :],
                            in_=onm[:])

        # ---------------- phase 3: GATv2 layer 1 -> y1T_loc ----------
        if PHASES >= 3:
         with (tc.tile_pool(name="p3s", bufs=3) as pool,
              tc.tile_pool(name="p3a", bufs=2, space="PSUM") as ppa,
              tc.tile_pool(name="p3t", bufs=2, space="PSUM") as ppt):
            with tc.For_i(0, NBLK, 1) as b:
                stg_s = pool.tile([128, T], I32, tag="stg_s")
                stg_d = pool.tile([128, T], FP, tag="stg_d")
                stg_c = pool.tile([128, T], FP, tag="stg_c")
                nc.sync.dma_start(out=stg_s[:], in_=srcs[:, ds(b * T, T)])
                nc.sync.dma_start(out=stg_d[:], in_=dstf[:, ds(b * T, T)])
                nc.sync.dma_start(out=stg_c[:], in_=cval[:, ds(b * T, T)])
                # xr_dense for this block: (x1_blk @ 0.6*W1r + 0.6*b1r), node-major
                x1b = pool.tile([64, 128], FP, tag="x1b")
                nc.sync.dma_start(out=x1b[:], in_=x1T_loc[:, ds(b * 128, 128)])
                psR1 = ppt.tile([128, 128], FP, tag="t128")
                nc.tensor.matmul(psR1[:], lhsT=ct['w1ra'][:], rhs=x1b[:],
                                 start=True, stop=True)
                psR2 = ppt.tile([16, 128], FP, tag="t128")
                nc.tensor.matmul(psR2[:], lhsT=ct['w1rb'][:], rhs=x1b[:],
                                 start=True, stop=True)
                sR1 = pool.tile([128, 128], FP, tag="sR1")
                nc.scalar.activation(sR1[:], psR1[:], AF.Identity,
                                     bias=ct['b1ra'][:, :1])
                sR2 = pool.tile([16, 128], FP, tag="sR2")
                nc.scalar.activation(sR2[:], psR2[:], AF.Identity,
                                     bias=ct['b1rb'][:, :1])
                tR1 = ppt.tile([128, 128], FP, tag="t128")
                nc.tensor.transpose(out=tR1[:], in_=sR1[:], identity=ct['ident'][:])
                tR2 = ppt.tile([128, 16], FP, tag="t128")
                nc.tensor.transpose(out=tR2[:], in_=sR2[:],
                                    identity=ct['ident'][:16, :16])
                xrd = pool.tile([128, F1], FP, tag="xrd")
                nc.scalar.activation(xrd[:, :128], tR1[:], AF.Copy)
                nc.scalar.activation(xrd[:, 128:F1], tR2[:], AF.Copy)

                blk = ppa.tile([128, F1 + H1], FP, tag="blk")
                for t in range(T):
                    gx = pool.tile([128, F1], FP, tag="gx")
                    nc.gpsimd.indirect_dma_start(
                        out=gx[:], out_offset=None, in_=xl1[:],
                        in_offset=bass.IndirectOffsetOnAxis(
                            ap=stg_s[:, t:t + 1], axis=0))
                    S = pool.tile([128, 128], FP, tag="S")
                    nc.vector.tensor_scalar(
                        S[:], ct['iota'][:], stg_d[:, t:t + 1],
                        stg_c[:, t:t + 1], op0=ALU.is_equal, op1=ALU.mult)
                    StP = ppt.tile([128, 128], FP, tag="t128")
                    nc.tensor.transpose(out=StP[:], in_=S[:], identity=ct['ident'][:])
                    St = pool.tile([128, 128], FP, tag="St")
                    nc.scalar.activation(St[:], StP[:], AF.Copy)
                    # z*0.6 = St @ xr_dense + 0.6*I @ gx
                    zP = ppt.tile([128, F1], FP, tag="zP")
                    nc.tensor.matmul(zP[:], lhsT=St[:], rhs=xrd[:],
                                     start=True, stop=False)
                    nc.tensor.matmul(zP[:], lhsT=ct['ident06'][:], rhs=gx[:],
                                     start=False, stop=True)
                    # lrelu(z, .2) = 0.6z + 0.4|z| = zP + |(2/3)*zP|
                    ab = pool.tile([128, F1], FP, tag="ab")
                    nc.scalar.activation(ab[:], zP[:], AF.Abs, scale=2.0 / 3.0)
                    lr = pool.tile([128, F1], FP, tag="lr")
                    nc.vector.tensor_tensor(out=lr[:], in0=ab[:], in1=zP[:],
                                            op=ALU.add)
                    wm = pool.tile([128, F1], FP, tag="wm")
                    nc.vector.tensor_tensor(out=wm[:], in0=lr[:],
                                            in1=ct['att1r'][:], op=ALU.mult)
                    lg = pool.tile([128, H1], FP, tag="lg")
                    nc.vector.tensor_reduce(
                        out=lg[:], in_=wm[:].rearrange("p (h c) -> p h c", c=C1),
                        axis=mybir.AxisListType.X, op=ALU.add)
                    ex = pool.tile([128, H1], FP, tag="ex")
                    nc.scalar.activation(ex[:], lg[:], AF.Exp, bias=ct['nshift'][:, :1])
                    rhs = pool.tile([128, F1 + H1], FP, tag="rhs")
                    nc.vector.tensor_tensor(
                        out=rhs[:, :F1].rearrange("p (h c) -> p h c", c=C1),
                        in0=gx[:].rearrange("p (h c) -> p h c", c=C1),
                        in1=ex[:].to_broadcast([128, H1, C1]), op=ALU.mult)
                    nc.vector.tensor_copy(out=rhs[:, F1:F1 + H1], in_=ex[:])
                    nc.tensor.matmul(blk[:], lhsT=S[:], rhs=rhs[:],
                                     start=(t == 0), stop=(t == T - 1))
                # normalize + bias, then transpose out
                den = pool.tile([128, H1], FP, tag="den")
                nc.vector.tensor_scalar(den[:], blk[:, F1:F1 + H1], 1e-16, None,
                                        op0=ALU.add)
                rec = pool.tile([128, H1], FP, tag="rec")
                nc.vector.reciprocal(rec[:], den[:])
                y1 = pool.tile([128, F1], FP, tag="y1")
                nc.vector.tensor_tensor(
                    out=y1[:].rearrange("p (h c) -> p h c", c=C1),
                    in0=blk[:, :F1].rearrange("p (h c) -> p h c", c=C1),
                    in1=rec[:].to_broadcast([128, H1, C1]), op=ALU.mult)
                tY1 = ppt.tile([128, 128], FP, tag="t128")
                nc.tensor.transpose(out=tY1[:], in_=y1[:, :128],
                                    identity=ct['ident'][:])
                tY2 = ppt.tile([16, 128], FP, tag="t128")
                nc.tensor.transpose(out=tY2[:], in_=y1[:, 128:F1],
                                    identity=ct['ident'][:])
                sY1 = pool.tile([128, 128], FP, tag="sY1")
                nc.scalar.activation(sY1[:], tY1[:], AF.Identity,
                                     bias=ct['bias1a'][:, :1])
                sY2 = pool.tile([16, 128], FP, tag="sY2")
                nc.scalar.activation(sY2[:], tY2[:], AF.Identity,
                                     bias=ct['bias1b'][:, :1])
                nc.sync.dma_start(out=y1T_loc[:128, ds(b * 128, 128)], in_=sY1[:])
                nc.sync.dma_start(out=y1T_loc[128:F1, ds(b * 128, 128)], in_=sY2[:])

        if PHASES >= 4:
         nc.gpsimd.collective_compute(
            "AllGather", ALU.bypass, replica_groups=rg,
            ins=[y1T_loc[:]], outs=[y1T_all[:]])

        # ---------------- phase 4: xl2 = y1 @ W2l + b2l --------------
        if PHASES >= 4:
         with (tc.tile_pool(name="p4s", bufs=3) as pool,
              tc.tile_pool(name="p4p", bufs=2, space="PSUM") as pp):
            with tc.For_i(0, CH, 1) as ci:
                for r in range(NCORES):
                    rhsT = pool.tile([128, 512], FP, tag="rhsT")
                    nc.sync.dma_start(
                        out=rhsT[:],
                        in_=y1T_all[r * F1:r * F1 + 128, ds(ci * 512, 512)])
                    rhsB = pool.tile([16, 512], FP, tag="rhsB")
                    nc.sync.dma_start(
                        out=rhsB[:],
                        in_=y1T_all[r * F1 + 128:(r + 1) * F1, ds(ci * 512, 512)])
                    psL = pp.tile([64, 512], FP, tag="psL")
                    nc.tensor.matmul(psL[:], lhsT=ct['w2lt'][:], rhs=rhsT[:],
                                     start=True, stop=False)
                    nc.tensor.matmul(psL[:], lhsT=ct['w2lb'][:], rhs=rhsB[:],
                                     start=False, stop=True)
                    sL = pool.tile([64, 512], FP, tag="sL")
                    nc.scalar.activation(sL[:], psL[:], AF.Identity,
                                         bias=ct['b2l'][:, :1])
                    for gi in range(4):
                        tp = pp.tile([128, 64], FP, tag="tp")
                        nc.tensor.transpose(out=tp[:],
                                            in_=sL[:, gi * 128:(gi + 1) * 128],
                                            identity=ct['ident'][:64, :64])
                        onm = pool.tile([128, F2], FP, tag="onm")
                        nc.scalar.activation(onm[:], tp[:], AF.Copy)
                        nc.sync.dma_start(
                            out=xl2[ds(r * PERCORE + ci * 512 + gi * 128, 128), :],
                            in_=onm[:])

        # ---------------- phase 5: GATv2 layer 2 -> out --------------
        if PHASES >= 5:
         with (tc.tile_pool(name="p5s", bufs=3) as pool,
              tc.tile_pool(name="p5a", bufs=2, space="PSUM") as ppa,
              tc.tile_pool(name="p5t", bufs=2, space="PSUM") as ppt):
            with tc.For_i(0, NBLK, 1) as b:
                stg_s = pool.tile([128, T], I32, tag="stg_s")
                stg_d = pool.tile([128, T], FP, tag="stg_d")
                stg_c = pool.tile([128, T], FP, tag="stg_c")
                nc.sync.dma_start(out=stg_s[:], in_=srcs[:, ds(b * T, T)])
                nc.sync.dma_start(out=stg_d[:], in_=dstf[:, ds(b * T, T)])
                nc.sync.dma_start(out=stg_c[:], in_=cval[:, ds(b * T, T)])
                y1b1 = pool.tile([128, 128], FP, tag="y1b1")
                nc.sync.dma_start(out=y1b1[:], in_=y1T_loc[:128, ds(b * 128, 128)])
                y1b2 = pool.tile([16, 128], FP, tag="y1b2")
                nc.sync.dma_start(out=y1b2[:], in_=y1T_loc[128:F1, ds(b * 128, 128)])
                psR = ppt.tile([64, 128], FP, tag="t128")
                nc.tensor.matmul(psR[:], lhsT=ct['w2rt'][:], rhs=y1b1[:],
                                 start=True, stop=False)
                nc.tensor.matmul(psR[:], lhsT=ct['w2rb'][:], rhs=y1b2[:],
                                 start=False, stop=True)
                sR = pool.tile([64, 128], FP, tag="sR")
                nc.scalar.activation(sR[:], psR[:], AF.Identity, bias=ct['b2r'][:, :1])
                tR = ppt.tile([128, 64], FP, tag="t128")
                nc.tensor.transpose(out=tR[:], in_=sR[:],
                                    identity=ct['ident'][:64, :64])
                xrd = pool.tile([128, F2], FP, tag="xrd")
                nc.scalar.activation(xrd[:], tR[:], AF.Copy)

                blk = ppa.tile([128, F2 + 1], FP, tag="blk")
                for t in range(T):
                    gx = pool.tile([128, F2], FP, tag="gx")
                    nc.gpsimd.indirect_dma_start(
                        out=gx[:], out_offset=None, in_=xl2[:],
                        in_offset=bass.IndirectOffsetOnAxis(
                            ap=stg_s[:, t:t + 1], axis=0))
                    S = pool.tile([128, 128], FP, tag="S")
                    nc.vector.tensor_scalar(
                        S[:], ct['iota'][:], stg_d[:, t:t + 1],
                        stg_c[:, t:t + 1], op0=ALU.is_equal, op1=ALU.mult)
                    StP = ppt.tile([128, 128], FP, tag="t128")
                    nc.tensor.transpose(out=StP[:], in_=S[:], identity=ct['ident'][:])
                    St = pool.tile([128, 128], FP, tag="St")
                    nc.scalar.activation(St[:], StP[:], AF.Copy)
                    zP = ppt.tile([128, F2], FP, tag="zP")
                    nc.tensor.matmul(zP[:], lhsT=St[:], rhs=xrd[:],
                                     start=True, stop=False)
                    nc.tensor.matmul(zP[:], lhsT=ct['ident06'][:], rhs=gx[:],
                                     start=False, stop=True)
                    ab = pool.tile([128, F2], FP, tag="ab")
                    nc.scalar.activation(ab[:], zP[:], AF.Abs, scale=2.0 / 3.0)
                    lr = pool.tile([128, F2], FP, tag="lr")
                    nc.vector.tensor_tensor(out=lr[:], in0=ab[:], in1=zP[:],
                                            op=ALU.add)
                    wm = pool.tile([128, F2], FP, tag="wm")
                    nc.vector.tensor_tensor(out=wm[:], in0=lr[:],
                                            in1=ct['att2r'][:], op=ALU.mult)
                    lg = pool.tile([128, 1], FP, tag="lg")
                    nc.vector.tensor_reduce(
                        out=lg[:], in_=wm[:].rearrange("p (h c) -> p h c", c=F2),
                        axis=mybir.AxisListType.X, op=ALU.add)
                    ex = pool.tile([128, 1], FP, tag="ex")
                    nc.scalar.activation(ex[:], lg[:], AF.Exp, bias=ct['nshift'][:, :1])
                    rhs = pool.tile([128, F2 + 1], FP, tag="rhs")
                    nc.vector.tensor_tensor(
                        out=rhs[:, :F2].rearrange("p (h c) -> p h c", c=F2),
                        in0=gx[:].rearrange("p (h c) -> p h c", c=F2),
                        in1=ex[:].to_broadcast([128, 1, F2]), op=ALU.mult)
                    nc.vector.tensor_copy(out=rhs[:, F2:F2 + 1], in_=ex[:])
                    nc.tensor.matmul(blk[:], lhsT=S[:], rhs=rhs[:],
                                     start=(t == 0), stop=(t == T - 1))
                den = pool.tile([128, 1], FP, tag="den")
                nc.vector.tensor_scalar(den[:], blk[:, F2:F2 + 1], 1e-16, None,
                                        op0=ALU.add)
                rec = pool.tile([128, 1], FP, tag="rec")
                nc.vector.reciprocal(rec[:], den[:])
                o1 = pool.tile([128, F2], FP, tag="o1")
                nc.vector.tensor_tensor(
                    out=o1[:].rearrange("p (h c) -> p h c", c=F2),
                    in0=blk[:, :F2].rearrange("p (h c) -> p h c", c=F2),
                    in1=rec[:].to_broadcast([128, 1, F2]), op=ALU.mult)
                o2 = pool.tile([128, F2], FP, tag="o2")
                nc.vector.tensor_tensor(out=o2[:], in0=o1[:], in1=ct['bias2r'][:],
                                        op=ALU.add)
                nc.sync.dma_start(out=out[ds(b * 128, 128), :], in_=o2[:])

        if PHASES < 5:
            with tc.tile_pool(name="zf", bufs=1) as zp:
                zt = zp.tile([128, F2], FP, tag="zt", name="zt")
                nc.vector.memset(zt[:], 0.0)
                with tc.For_i(0, NBLK, 1) as b:
                    nc.sync.dma_start(out=out[ds(b * 128, 128), :], in_=zt[:])
        cpool.release()

    nc.compile()
    _split_sync_waits(nc)
    return nc


_NC_CACHE = {}


def kernel(**inputs):
    features = inputs["features"]
    edge_index = inputs["edge_index"]
    in_maps, newid, T = _host_prep(features, edge_index, inputs)
    if T not in _NC_CACHE:
        _NC_CACHE[T] = _build(T)
    nc = _NC_CACHE[T]
    res = run_bass_kernel_spmd(nc, in_maps, list(range(NCORES)))
    y_new = np.concatenate([res.results[c]["out"] for c in range(NCORES)], axis=0)
    return np.ascontiguousarray(y_new[newid[:N]]).astype(np.float32)

